# revision 7
# baseline (speedup 1.0000x reference)
"""Trainium2 Bass kernel for nn_AU_Net_3573412790684 (GNN message passing), v2.

Strategy (8 NeuronCores, SPMD + collectives):
  - Node dim padded 1026 -> NP=1152 (9*128); nodes sharded 144/core.
  - All weight/activation matmul streams bf16 (fp32 psum/bias/state);
    halves HBM + collective bytes vs fp32 at identical PE speed.
  - GDC exact PPR via 7-step Neumann doubling (128 terms) on row-sharded G;
    gathered G cached in SBUF once per step; chain collectives in bf16.
  - ec1's xn-half restructured via associativity: z_xn = zz @ (x @ W1x),
    removing the xn stage and two 18.9MB AllGathers entirely.
  - ahatT (GCN aggregation matrix) cached in SBUF bf16, reused by both GCN
    layers; snT (normalized diffusion) cached likewise for the z_xn matmul.
  - Independent GEMM chunks (ec1-gx, xw) interleaved into the chain's
    AllGather gaps to keep PE busy.
  - Collective in/out tensors pinned to unique unpooled DRAM (recycled
    Shared buffers break remote writes on this runtime).
"""
import sys
import os
import numpy as np

sys.path.insert(0, "/opt/trn_rl_repo")
import concourse.bass as bass
from concourse import bacc
import concourse.mybir as mybir
import concourse.tile as tile
from concourse import bass_utils
import ml_dtypes

import bass_rust

_SKIP_WAIT_SPLIT = ("InstDrain", "InstCollectiveCompute", "InstEventSemaphore",
                    "InstCall", "InstHalt", "InstAllEngineBarrier",
                    "InstBranchHint")
_ev_uid = [0]


def legalize_matmul_waits(nc, max_waits: int = 1):
    """walrus rejects instructions carrying more than one sync-wait command;
    split excess waits into standalone same-engine InstEventSemaphores."""
    moved = 0
    for f in nc.m.functions:
        for bb in f.blocks:
            out = []
            for ins in bb.instructions:
                tn = type(ins).__name__
                si = ins.sync_info
                if (si is not None and len(si.on_wait) > max_waits
                        and tn not in _SKIP_WAIT_SPLIT):
                    for w in list(si.on_wait):
                        _ev_uid[0] += 1
                        ev = mybir.InstEventSemaphore(
                            name=f"waitev-{_ev_uid[0]}", ins=[], outs=[])
                        ev.engine = ins.engine
                        ev.sync_info = bass_rust.SyncInfo(on_wait=[w], on_update=[])
                        ev.bass_nofuse = True
                        out.append(ev)
                    ins.sync_info = bass_rust.SyncInfo(
                        on_wait=[], on_update=list(si.on_update))
                    moved += 1
                out.append(ins)
            bb.instructions[:] = out
    return moved

F32 = mybir.dt.float32
F32R = mybir.dt.float32r
BF16 = mybir.dt.bfloat16
AF = mybir.ActivationFunctionType

N = 1026
NP = 1152
S = NP // 8
DX = 4096
INS = 8192
JH = 2048
H0 = 4096
H1 = 2048
H2 = 1024
OUTS = 512
NL = 10
TOPK = 128
CHAIN_ITERS = int(os.environ.get("CHAIN_ITERS", "6"))  # 2^n Neumann terms
NCORES = 8
NC3 = [(0, 384), (384, 384), (768, 384)]     # full width (GDC chain)
NCF = [(0, 384), (384, 384), (768, 258)]     # feature gemms: skip pad cols
BLKS = [(0, 0, 128), (1, 128, 16)]

PS_TAGS = ["pA", "pA", "pB", "pC"]           # tp_gemm m-tile psum tags
PS_BUFS = [2, 2, 1, 1]


def _ceil(a, b):
    return -(-a // b)


def _mtiles(M):
    out, o = [], 0
    while o < M:
        t = min(128, M - o)
        out.append((o, t))
        o += t
    return out


class Prog:
    def __init__(self):
        self.nc = bacc.Bacc("TRN2", target_bir_lowering=False, debug=False,
                            num_devices=NCORES)
        self.uid = 0

    def name(self, p):
        self.uid += 1
        return f"{p}_{self.uid}"


def bv(t, bi, n_off=0, n_sz=NP, rows=None):
    r = (128 if bi == 0 else 16) if rows is None else rows
    return t[0:r, bi * NP + n_off: bi * NP + n_off + n_sz]


def tp_gemm(P, sb, ps, kxm_srcs, kxn_srcs, M, epilogue, n_chunks=NCF,
            dt=BF16, kxm_tag="kxmC"):
    """out[M, n] = kxm^T @ kxn, chunked over n; psum f32; streams dtype dt."""
    nc = P.nc
    ktiles = []
    for si, (ap, rows) in enumerate(kxm_srcs):
        for r in range(0, rows, 128):
            ktiles.append((si, r))
    nkt = len(ktiles)
    rh = []
    for si, (ap, rows) in enumerate(kxn_srcs):
        for r in range(0, rows, 128):
            rh.append((si, r))
    assert len(rh) == nkt
    mts = _mtiles(M)

    kxm_sb = sb.tile([128, nkt * M], dt, name=P.name("kxmC"), tag=kxm_tag)
    kt = 0
    while kt < nkt:
        si, r = ktiles[kt]
        nb = 1
        while (nb < 4 and kt + nb < nkt and ktiles[kt + nb][0] == si
               and ktiles[kt + nb][1] == r + nb * 128):
            nb += 1
        nc.scalar.dma_start(
            kxm_sb[:, kt * M:(kt + nb) * M],
            kxm_srcs[si][0][r:r + nb * 128, :].rearrange("(a p) m -> p a m", p=128))
        kt += nb

    # batch consecutive k-tiles of the same rhs source into one DMA
    KB = 8
    batches = []
    kt = 0
    while kt < nkt:
        si, r = rh[kt]
        nb = 1
        while (nb < KB and kt + nb < nkt and rh[kt + nb][0] == si
               and rh[kt + nb][1] == r + nb * 128):
            nb += 1
        batches.append((si, r, nb, kt))
        kt += nb

    if len(mts) <= 2:
        # all chunks' psums alive at once; lhsT loaded once per (kt, mi)
        PTAGS2 = [["pA", "pA"], ["pB", "pC"], ["pD", "pE"]]
        PBUFS2 = {"pA": 2, "pB": 1, "pC": 1, "pD": 1, "pE": 1}
        psums_all = [[ps.tile([m_sz, n_sz], F32, name=P.name("psg"),
                              tag=PTAGS2[ci][mi], bufs=PBUFS2[PTAGS2[ci][mi]])
                      for mi, (m_off, m_sz) in enumerate(mts)]
                     for ci, (n_off, n_sz) in enumerate(n_chunks)]
        for (si, r, nb, kt0) in batches:
            rts = []
            for ci, (n_off, n_sz) in enumerate(n_chunks):
                rt = sb.tile([128, KB * n_sz], dt, name=P.name("rhs"),
                             tag="rhs", bufs=3)
                src = kxn_srcs[si][0][r:r + nb * 128, n_off:n_off + n_sz]
                nc.sync.dma_start(rt[0:128, 0:nb * n_sz],
                                  src.rearrange("(a p) n -> p a n", p=128))
                rts.append(rt)
            for kk in range(nb):
                kt = kt0 + kk
                for mi, (m_off, m_sz) in enumerate(mts):
                    lh = kxm_sb[:, kt * M + m_off: kt * M + m_off + m_sz]
                    for ci, (n_off, n_sz) in enumerate(n_chunks):
                        nc.tensor.matmul(psums_all[ci][mi][:], lh,
                                         rts[ci][:, kk * n_sz:(kk + 1) * n_sz],
                                         start=(kt == 0), stop=(kt == nkt - 1))
        for ci, (n_off, n_sz) in enumerate(n_chunks):
            for mi, (m_off, m_sz) in enumerate(mts):
                epilogue(mi, m_off, m_sz, n_off, n_sz, psums_all[ci][mi])
    else:
        for ci, (n_off, n_sz) in enumerate(n_chunks):
            psums = [ps.tile([m_sz, n_sz], F32, name=P.name("psg"),
                             tag=PS_TAGS[mi], bufs=PS_BUFS[mi])
                     for mi, (m_off, m_sz) in enumerate(mts)]
            for (si, r, nb, kt0) in batches:
                rt = sb.tile([128, KB * n_sz], dt, name=P.name("rhs"), tag="rhs", bufs=3)
                src = kxn_srcs[si][0][r:r + nb * 128, n_off:n_off + n_sz]
                nc.sync.dma_start(rt[0:128, 0:nb * n_sz],
                                  src.rearrange("(a p) n -> p a n", p=128))
                for kk in range(nb):
                    kt = kt0 + kk
                    for mi, (m_off, m_sz) in enumerate(mts):
                        lh = kxm_sb[:, kt * M + m_off: kt * M + m_off + m_sz]
                        nc.tensor.matmul(psums[mi][:], lh,
                                         rt[:, kk * n_sz:(kk + 1) * n_sz],
                                         start=(kt == 0), stop=(kt == nkt - 1))
            for mi, (m_off, m_sz) in enumerate(mts):
                epilogue(mi, m_off, m_sz, n_off, n_sz, psums[mi])


def act_epilogue(P, sb, out_dram, bias_tile, func, out_sb_fn=None, dt=BF16):
    nc = P.nc

    def ep(mi, m_off, m_sz, n_off, n_sz, psum):
        t = sb.tile([m_sz, n_sz], dt, name=P.name("ep"), tag="ep", bufs=3)
        if bias_tile is not None and func == AF.Copy:
            nc.vector.tensor_scalar_add(t[:], psum[:], bias_tile[0:m_sz, mi:mi + 1])
        elif bias_tile is not None:
            nc.scalar.activation(t[:], psum[:], func,
                                 bias=bias_tile[0:m_sz, mi:mi + 1])
        else:
            nc.scalar.activation(t[:], psum[:], func)
        if out_dram is not None:
            nc.scalar.dma_start(out_dram[m_off:m_off + m_sz, n_off:n_off + n_sz], t[:])
        if out_sb_fn is not None:
            nc.vector.tensor_copy(out_sb_fn(mi, m_off, m_sz, n_off, n_sz), t[:])
    return ep


def load_bias(P, sb, bias_dram, M):
    nc = P.nc
    t = sb.tile([128, _ceil(M, 128)], F32, name=P.name("bias"),
                tag=P.name("bias"), bufs=1)
    for mi, (m_off, m_sz) in enumerate(_mtiles(M)):
        nc.scalar.dma_start(t[:m_sz, mi:mi + 1], bias_dram[m_off:m_off + m_sz, :])
    return t


def build_program():
    P = Prog()
    nc = P.nc

    def inp(name, shape, dt=BF16):
        return nc.dram_tensor(name, shape, dt, kind="ExternalInput")

    xgT = inp("xgT", [INS, NP])
    eyeT = inp("eyeT", [S, NP], F32R)
    vmask = inp("vmask", [1, NP], F32)
    ahatT = inp("ahatT", [NP, NP])
    w_jw1 = inp("w_jw1", [INS, JH // 8]); b_jb1 = inp("b_jb1", [JH // 8, 1], F32)
    w_jw2 = inp("w_jw2", [JH, S]); b_jb2 = inp("b_jb2", [S, 1], F32)
    w_ec1x = inp("w_ec1x", [DX, H0 // 8])
    w_ec1g = inp("w_ec1g", [DX, H0 // 8]); b_ec1 = inp("b_ec1", [H0 // 8, 1], F32)
    w_zp = inp("w_zp", [H0, 512])            # [dr_w | ec2_w[:DX] | g1_w]
    b_dr = inp("b_dr", [H2 // 8, 1], F32)
    w_g1gx = inp("w_g1gx", [DX, H1 // 8])
    b_g1 = inp("b_g1", [H1 // 8, 1], F32)
    w_z1p = inp("w_z1p", [H1, 256])          # [ec2_w[DX:DX+H1] | g2_w]
    b_g2 = inp("b_g2", [H2 // 8, 1], F32)
    w_ec2c = inp("w_ec2c", [H2, H2 // 8]); b_ec2 = inp("b_ec2", [H2 // 8, 1], F32)
    w_ec3 = inp("w_ec3", [H2, OUTS // 8]); b_ec3 = inp("b_ec3", [OUTS // 8, 1], F32)
    w_out = inp("w_out", [OUTS, NL]); b_out = inp("b_out", [NL, 1], F32)
    identR = inp("identR", [128, 128], F32R)
    onescol = inp("onescol", [128, 1], F32R)
    onesrow = inp("onesrow", [1, 128], F32R)

    outT = nc.dram_tensor("outT", [NL, NP], F32, kind="ExternalOutput")

    def shared(name, shape, dt=BF16):
        return nc.dram_tensor(name, shape, dt, kind="Internal",
                              addr_space="Shared")

    def local(name, shape, dt=BF16):
        return nc.dram_tensor(name, shape, dt, kind="Internal")

    with tile.TileContext(nc) as tc:
        with tc.tile_pool(name="sb", bufs=1) as sb, \
             tc.tile_pool(name="ps", bufs=1, space="PSUM") as ps:

            ident = sb.tile([128, 128], F32R, name="ident")
            nc.sync.dma_start(ident[:], identR[:])

            zrow = sb.tile([128, NP - N], BF16, name="zrow")
            zrow_f = sb.tile([128, NP - N], F32, name="zrow_f")
            nc.vector.memset(zrow_f[:], 0.0)
            nc.vector.tensor_copy(zrow[:], zrow_f[:])

            def zero_pads(dram_t, rows):
                for mo in range(0, rows, 128):
                    msz = min(128, rows - mo)
                    nc.scalar.dma_start(dram_t[mo:mo + msz, N:NP], zrow[0:msz, :])

            def transpose_block(src_ap, pt_shape, dst_ap):
                pt = ps.tile(pt_shape, F32R, name=P.name("ptr"), tag="tr", bufs=2)
                idn = ident[0:pt_shape[1], 0:pt_shape[1]]
                nc.tensor.transpose(pt[:], src_ap, idn)
                nc.vector.tensor_copy(dst_ap, pt[:])

            FAKE_CC = os.environ.get("FAKE_CC", "0") == "1"

            def cc_or_fake(slice_dram, full, rows):
                if FAKE_CC:
                    for c in range(NCORES):
                        nc.gpsimd.dma_start(full[c * rows:(c + 1) * rows, :],
                                            slice_dram[:, :])
                else:
                    nc.gpsimd.collective_compute(
                        "AllGather", mybir.AluOpType.bypass,
                        replica_groups=[list(range(NCORES))],
                        ins=[slice_dram[:, :].opt()], outs=[full[:, :].opt()])

            def allgather(slice_dram, full_shape, name, dt=BF16):
                full = shared(name, full_shape, dt)
                cc_or_fake(slice_dram, full, full_shape[0] // NCORES)
                return full

            # ============ A: zz1 = relu(xg @ jw1) ============
            zz1_sl = local("zz1_sl", [JH // 8, NP])
            zero_pads(zz1_sl, JH // 8)
            bt = load_bias(P, sb, b_jb1, JH // 8)
            tp_gemm(P, sb, ps, [(w_jw1, INS)], [(xgT, INS)], JH // 8,
                    act_epilogue(P, sb, zz1_sl, bt, AF.Relu))
            zz1_full = allgather(zz1_sl, [JH, NP], "zz1_full")

            # ============ B: zzT slice (zz^T rows = my cols of zz) ============
            zzT = sb.tile([128, 2 * NP], F32R, name="zzT", tag="gxpart")
            zzpad = sb.tile([128, NP - N], F32, name="zzpad")
            nc.vector.memset(zzpad[:], 0.0)
            bt2 = load_bias(P, sb, b_jb2, S)

            def zz_out(mi, m_off, m_sz, n_off, n_sz):
                return bv(zzT, mi, n_off, n_sz, rows=m_sz)
            tp_gemm(P, sb, ps, [(w_jw2, JH)], [(zz1_full, JH)], S,
                    act_epilogue(P, sb, None, bt2, AF.Relu, out_sb_fn=zz_out))
            for bi, ro, rs in BLKS:
                nc.vector.tensor_copy(bv(zzT, bi, N, NP - N), zzpad[0:rs, :])

            # ============ C: deg / dinv ============
            ones_sl = sb.tile([128, 1], F32R, name="ones_sl")
            nc.sync.dma_start(ones_sl[:], onescol[:])
            deg_sb = sb.tile([1, NP], F32, name="deg_sb")
            for (n_off, n_sz) in NC3:
                dps = ps.tile([1, n_sz], F32, name=P.name("dps"), tag="tr", bufs=2)
                nc.tensor.matmul(dps[:], ones_sl[0:128, :], bv(zzT, 0, n_off, n_sz),
                                 start=True, stop=False)
                nc.tensor.matmul(dps[:], ones_sl[0:16, :], bv(zzT, 1, n_off, n_sz),
                                 start=False, stop=True)
                nc.vector.tensor_copy(deg_sb[:, n_off:n_off + n_sz], dps[:])
            deg_bin = local("deg_bin", [1, NP], F32)
            nc.gpsimd.dma_start(deg_bin[:, :], deg_sb[:])
            deg_full = shared("deg_full", [1, NP], F32)
            if FAKE_CC:
                nc.gpsimd.dma_start(deg_full[:, :], deg_bin[:, :])
            else:
                nc.gpsimd.collective_compute(
                    "AllReduce", mybir.AluOpType.add,
                    replica_groups=[list(range(NCORES))],
                    ins=[deg_bin[:, :].opt()], outs=[deg_full[:, :].opt()])
            dinv_f = sb.tile([1, NP], F32, name="dinv_f")
            vm = sb.tile([1, NP], F32, name="vm")
            nc.sync.dma_start(vm[:], vmask[:])
            nc.sync.dma_start(dinv_f[:], deg_full[:, :])
            nc.vector.tensor_scalar_add(dinv_f[:], dinv_f[:], 1.0)
            nc.vector.reciprocal(dinv_f[:], dinv_f[:])
            nc.scalar.activation(dinv_f[:], dinv_f[:], AF.Sqrt)
            nc.vector.tensor_mul(dinv_f[:], dinv_f[:], vm[:])

            onesr = sb.tile([1, 128], F32R, name="onesr")
            nc.sync.dma_start(onesr[:], onesrow[:])
            dinv_fr = sb.tile([1, NP], F32R, name="dinv_fr")
            nc.vector.tensor_copy(dinv_fr[:], dinv_f[:])
            dinv_b = sb.tile([128, NP], F32R, name="dinv_b", tag="hT")
            for (n_off, n_sz) in NC3:
                bps = ps.tile([128, n_sz], F32, name=P.name("bps"), tag="tr", bufs=2)
                nc.tensor.matmul(bps[:], onesr[:], dinv_fr[:, n_off:n_off + n_sz],
                                 start=True, stop=True)
                nc.vector.tensor_copy(dinv_b[:, n_off:n_off + n_sz], bps[:])

            eyeT_sb = sb.tile([128, 2 * NP], F32R, name="eyeT_sb", tag="h1sb")
            nc.sync.dma_start(bv(eyeT_sb, 0), eyeT[0:128, :])
            nc.sync.dma_start(bv(eyeT_sb, 1), eyeT[128:S, :])
            dinv_p = sb.tile([128, 2], F32, name="dinv_p")
            tmpm = sb.tile([128, NP], F32R, name="tmpm", tag="scratch")
            for bi, ro, rs in BLKS:
                nc.vector.tensor_mul(tmpm[0:rs, :], bv(eyeT_sb, bi), dinv_b[0:rs, :])
                nc.vector.reduce_sum(dinv_p[0:rs, bi:bi + 1], tmpm[0:rs, :],
                                     axis=mybir.AxisListType.X)

            # ============ D: G slice + V init ============
            g_sl = sb.tile([128, 2 * NP], F32R, name="g_sl0")
            v_sl = sb.tile([128, 2 * NP], F32R, name="v_sl0")
            for bi, ro, rs in BLKS:
                g = bv(g_sl, bi)
                nc.vector.tensor_add(g, bv(zzT, bi), bv(eyeT_sb, bi))
                nc.vector.tensor_scalar_mul(g, g, dinv_p[0:rs, bi:bi + 1])
                nc.vector.tensor_mul(g, g, dinv_b[0:rs, :])
                nc.vector.tensor_scalar_mul(g, g, 0.95)
                nc.vector.tensor_add(bv(v_sl, bi), bv(eyeT_sb, bi), g)

            # ====== g1gx: gx part of GCN1 pre-agg (before chain) ======
            W1 = H1 // 8
            gxpart = sb.tile([128, 2 * NP], F32R, name="gxpart", tag="gxpart")
            g1x_sb = sb.tile([128, 32 * W1], BF16, name="g1x_sb", tag="kxmC")
            for kt in range(0, 32, 4):
                nc.sync.dma_start(
                    g1x_sb[:, kt * W1:(kt + 4) * W1],
                    w_g1gx[kt * 128:(kt + 4) * 128, :]
                    .rearrange("(a p) m -> p a m", p=128))
            for (n_off, n_sz) in NCF:
                pgx = [ps.tile([128, n_sz], F32, name=P.name("pgx"), tag="pA", bufs=2)
                       for _ in range(2)]
                for bb in range(8):
                    rt4 = sb.tile([128, 4 * n_sz], BF16, name=P.name("gxr"),
                                  tag="rhs", bufs=3)
                    nc.sync.dma_start(
                        rt4[:],
                        xgT[DX + bb * 512: DX + (bb + 1) * 512, n_off:n_off + n_sz]
                        .rearrange("(a p) n -> p a n", p=128))
                    for kk in range(4):
                        kt = bb * 4 + kk
                        rt = rt4[:, kk * n_sz:(kk + 1) * n_sz]
                        for i in range(2):
                            nc.tensor.matmul(
                                pgx[i][:],
                                g1x_sb[:, kt * W1 + i * 128: kt * W1 + i * 128 + 128],
                                rt, start=(kt == 0), stop=(kt == 31))
                for i in range(2):
                    nc.vector.tensor_copy(
                        gxpart[0:128, i * NP + n_off: i * NP + n_off + n_sz], pgx[i][:])

            # ====== interleaved piece machinery: ec1-gx + xw chunks ======
            # zpart_sb: (gx @ W1g)^T slices [4 m-tiles, NP]; xw_lhsT: transposed
            # (x @ W1x) node-major lhsT tiles for the snT matmul.
            zpart_sb = sb.tile([128, 4 * NP], BF16, name="zpart_sb", tag="zpart")
            xw_lhsT = sb.tile([128, 36 * 128], BF16, name="xw_lhsT", tag="xwl")
            nc.vector.memset(xw_lhsT[:], 0.0)
            def piece(kind, ci):
                n_off, n_sz = NCF[ci]
                wsrc = w_ec1g if kind == "zp" else w_ec1x
                xoff = DX if kind == "zp" else 0
                psums = [ps.tile([128, n_sz], F32, name=P.name("pp"),
                                 tag=PS_TAGS[mi], bufs=PS_BUFS[mi])
                         for mi in range(4)]
                for bb in range(8):
                    wt4 = sb.tile([128, 4 * 512], BF16, name=P.name("pw"),
                                  tag="wstr", bufs=2)
                    nc.scalar.dma_start(
                        wt4[:],
                        wsrc[bb * 512:(bb + 1) * 512, :]
                        .rearrange("(a p) m -> p a m", p=128))
                    rt4 = sb.tile([128, 4 * n_sz], BF16, name=P.name("pr"),
                                  tag="rhs", bufs=3)
                    nc.sync.dma_start(
                        rt4[:],
                        xgT[xoff + bb * 512: xoff + (bb + 1) * 512,
                            n_off:n_off + n_sz]
                        .rearrange("(a p) n -> p a n", p=128))
                    for kk in range(4):
                        kt = bb * 4 + kk
                        rt = rt4[:, kk * n_sz:(kk + 1) * n_sz]
                        for mi in range(4):
                            nc.tensor.matmul(
                                psums[mi][:],
                                wt4[:, kk * 512 + mi * 128: kk * 512 + (mi + 1) * 128],
                                rt, start=(kt == 0), stop=(kt == 31))
                if kind == "zp":
                    for mi in range(4):
                        nc.vector.tensor_copy(
                            zpart_sb[:, mi * NP + n_off: mi * NP + n_off + n_sz],
                            psums[mi][:])
                else:
                    # transpose xwT [128 f, n] chunks into node-major lhsT tiles
                    assert n_off % 128 == 0
                    kb0 = n_off // 128
                    nkb = _ceil(n_sz, 128)
                    for mi in range(4):
                        stg = sb.tile([128, n_sz], F32R, name=P.name("xwst"),
                                      tag="ep", bufs=3)
                        nc.vector.tensor_copy(stg[:], psums[mi][:])
                        for kk in range(nkb):
                            cw = min(128, n_sz - kk * 128)
                            transpose_block(
                                stg[0:128, kk * 128: kk * 128 + cw],
                                [cw, 128],
                                xw_lhsT[0:cw,
                                        (mi * 9 + kb0 + kk) * 128:
                                        (mi * 9 + kb0 + kk) * 128 + 128])

            PIECES = [("zp", 0), ("zp", 1), ("zp", 2),
                      ("xw", 0), ("xw", 1), ("xw", 2)]
            PIECES_IN_CHAIN = os.environ.get("PIECES_IN_CHAIN", "1") == "1"
            if not PIECES_IN_CHAIN:
                for pc in PIECES:
                    piece(*pc)

            # ============ E: doubling chain (7 steps, bf16 wire) ============
            gT = sb.tile([128, 9 * S], BF16, name="gT")
            vT = sb.tile([128, 9 * S], BF16, name="vT")

            def transpose_slice(src_bt, dst_sb):
                for kb in range(9):
                    transpose_block(bv(src_bt, 0, kb * 128, 128), [128, 128],
                                    dst_sb[:, kb * S: kb * S + 128])
                    transpose_block(bv(src_bt, 1, kb * 128, 128), [128, 16],
                                    dst_sb[:, kb * S + 128: (kb + 1) * S])

            for j in range(1, CHAIN_ITERS + 1):
                last = (j == CHAIN_ITERS)
                transpose_slice(g_sl, gT)
                if j > 1:
                    transpose_slice(v_sl, vT)
                gb = local(P.name("g_bin"), [S, NP])
                nc.gpsimd.dma_start(gb[0:128, :], bv(g_sl, 0))
                nc.gpsimd.dma_start(gb[128:S, :], bv(g_sl, 1))
                g_full = shared(P.name("g_full"), [NP, NP])
                cc_or_fake(gb, g_full, S)
                for (n_off, n_sz) in NC3:
                    pg0 = pg1 = pv0 = pv1 = None
                    if not last:
                        pg0 = ps.tile([128, n_sz], F32, name=P.name("pg0"), tag="pB", bufs=1)
                        pg1 = ps.tile([16, n_sz], F32, name=P.name("pg1"), tag="pC", bufs=1)
                    if j > 1:
                        pv0 = ps.tile([128, n_sz], F32, name=P.name("pv0"), tag="pD", bufs=1)
                        pv1 = ps.tile([16, n_sz], F32, name=P.name("pv1"), tag="pE", bufs=1)
                    rt9 = sb.tile([128, 9 * n_sz], BF16, name=P.name("grhs"),
                                  tag="grhs", bufs=2)
                    nc.sync.dma_start(
                        rt9[:],
                        g_full[0:NP, n_off:n_off + n_sz]
                        .rearrange("(a p) n -> p a n", p=128))
                    for kb in range(9):
                        rt = rt9[:, kb * n_sz:(kb + 1) * n_sz]
                        st, sp = (kb == 0), (kb == 8)
                        if not last:
                            nc.tensor.matmul(pg0[:], gT[:, kb * S: kb * S + 128],
                                             rt, start=st, stop=sp)
                            nc.tensor.matmul(pg1[:], gT[:, kb * S + 128:(kb + 1) * S],
                                             rt, start=st, stop=sp)
                        if j > 1:
                            nc.tensor.matmul(pv0[:], vT[:, kb * S: kb * S + 128],
                                             rt, start=st, stop=sp)
                            nc.tensor.matmul(pv1[:], vT[:, kb * S + 128:(kb + 1) * S],
                                             rt, start=st, stop=sp)
                    pgs, pvs = [pg0, pg1], [pv0, pv1]
                    for bi, ro, rs in BLKS:
                        if j > 1:
                            nc.vector.tensor_add(bv(v_sl, bi, n_off, n_sz),
                                                 bv(v_sl, bi, n_off, n_sz), pvs[bi][:])
                        if not last:
                            nc.vector.tensor_copy(bv(g_sl, bi, n_off, n_sz), pgs[bi][:])
                if PIECES_IN_CHAIN and j <= len(PIECES):
                    piece(*PIECES[j - 1])

            # ============ F: topk + column normalize ============
            vf = sb.tile([128, 2 * NP], F32, name="vf", tag="hT")
            work = sb.tile([128, 2 * NP], F32, name="tkwork", tag="scratch")
            mx = sb.tile([128, 8], F32, name="tkmax")
            for bi, ro, rs in BLKS:
                nc.vector.tensor_copy(bv(vf, bi), bv(v_sl, bi))
            for bi, ro, rs in BLKS:
                cur = bv(vf, bi)
                w = bv(work, bi)
                for it in range(TOPK // 8):
                    nc.vector.max(mx[0:rs, :], cur)
                    nc.vector.match_replace(w, mx[0:rs, :], cur, 0.0)
                    cur = w
            csum = sb.tile([128, 2], F32, name="csum")
            for bi, ro, rs in BLKS:
                nc.vector.tensor_sub(bv(work, bi), bv(vf, bi), bv(work, bi))
                nc.vector.reduce_sum(csum[0:rs, bi:bi + 1], bv(work, bi),
                                     axis=mybir.AxisListType.X)
            nc.vector.tensor_scalar_add(csum[:], csum[:], 1e-30)
            nc.vector.reciprocal(csum[:], csum[:])
            for bi, ro, rs in BLKS:
                nc.vector.tensor_scalar_mul(bv(work, bi), bv(work, bi),
                                            csum[0:rs, bi:bi + 1])
            sn_bin = local("sn_bin", [S, NP])
            nc.gpsimd.dma_start(sn_bin[0:128, :], bv(work, 0))
            nc.gpsimd.dma_start(sn_bin[128:S, :], bv(work, 1))
            snT_full = shared("snT_full", [NP, NP])
            cc_or_fake(sn_bin, snT_full, S)

            def stream3(src_dram, n_off, n_sz):
                rt9 = sb.tile([128, 9 * n_sz], BF16, name=P.name("str"),
                              tag="grhs", bufs=2)
                nc.sync.dma_start(
                    rt9[:],
                    src_dram[0:NP, n_off:n_off + n_sz]
                    .rearrange("(a p) n -> p a n", p=128))
                return lambda kb: rt9[:, kb * n_sz:(kb + 1) * n_sz]

            # ============ G: z = relu(zz@xw + gx-part + bias) ============
            bt_ec1 = load_bias(P, sb, b_ec1, H0 // 8)
            zT_sl = local("zT_sl", [H0 // 8, NP])
            zero_pads(zT_sl, H0 // 8)
            for (ci, (n_off, n_sz)) in enumerate(NCF):
                psums = [ps.tile([128, n_sz], F32, name=P.name("pz"),
                                 tag=PS_TAGS[mi], bufs=PS_BUFS[mi])
                         for mi in range(4)]
                rtf = stream3(snT_full, n_off, n_sz)
                for kb in range(9):
                    rt = rtf(kb)
                    for mi in range(4):
                        nc.tensor.matmul(
                            psums[mi][:],
                            xw_lhsT[:, (mi * 9 + kb) * 128:(mi * 9 + kb + 1) * 128],
                            rt, start=(kb == 0), stop=(kb == 8))
                for mi in range(4):
                    t = sb.tile([128, n_sz], F32R, name=P.name("zt"), tag="ep", bufs=3)
                    nc.vector.tensor_add(
                        t[:], psums[mi][:],
                        zpart_sb[:, mi * NP + n_off: mi * NP + n_off + n_sz])
                    tb = sb.tile([128, n_sz], BF16, name=P.name("ztb"), tag="ep", bufs=3)
                    nc.scalar.activation(tb[:], t[:], AF.Relu,
                                         bias=bt_ec1[0:128, mi:mi + 1])
                    nc.sync.dma_start(
                        zT_sl[mi * 128:(mi + 1) * 128, n_off:n_off + n_sz], tb[:])
            zT_full = allgather(zT_sl, [H0, NP], "zT_full")

            # ============ H: zpass [z0 | zc-z | h1-z] ============
            z0_sb = sb.tile([128, NP], F32R, name="z0_sb")
            zc_acc = sb.tile([128, NP], F32R, name="zc_acc")
            hT_tmp = sb.tile([128, 2 * NP], F32R, name="hT_tmp", tag="hT")
            bt_dr = load_bias(P, sb, b_dr, H2 // 8)

            def ep_zpass(mi, m_off, m_sz, n_off, n_sz, psum):
                if mi == 0:
                    nc.vector.tensor_scalar_add(z0_sb[:, n_off:n_off + n_sz], psum[:],
                                                bt_dr[:, 0:1])
                elif mi == 1:
                    nc.vector.tensor_copy(zc_acc[:, n_off:n_off + n_sz], psum[:])
                else:
                    bi = mi - 2
                    nc.vector.tensor_add(
                        hT_tmp[:, bi * NP + n_off: bi * NP + n_off + n_sz],
                        gxpart[:, bi * NP + n_off: bi * NP + n_off + n_sz], psum[:])
            tp_gemm(P, sb, ps, [(w_zp, H0)], [(zT_full, H0)], 512, ep_zpass)

            h1_sb = sb.tile([128, 9 * W1], BF16, name="h1_sb", tag="h1sb")
            zpad = sb.tile([128, NP - N], F32, name="zpad")
            nc.vector.memset(zpad[:], 0.0)
            nc.vector.tensor_copy(hT_tmp[:, N:NP], zpad[:])
            nc.vector.tensor_copy(hT_tmp[:, NP + N:2 * NP], zpad[:])
            for kb in range(9):
                for i in range(2):
                    transpose_block(
                        hT_tmp[0:128, i * NP + kb * 128: i * NP + (kb + 1) * 128],
                        [128, 128],
                        h1_sb[:, kb * W1 + i * 128: kb * W1 + (i + 1) * 128])

            # ============ I: z1 agg ============
            z1_sl = local("z1_sl", [H1 // 8, NP])
            zero_pads(z1_sl, H1 // 8)
            bt_g1 = load_bias(P, sb, b_g1, H1 // 8)
            ep_z1 = act_epilogue(P, sb, z1_sl, bt_g1, AF.Relu)
            for (n_off, n_sz) in NCF:
                pz0 = ps.tile([128, n_sz], F32, name=P.name("pz0"), tag="pA", bufs=2)
                pz1 = ps.tile([128, n_sz], F32, name=P.name("pz1"), tag="pA", bufs=2)
                rtf = stream3(ahatT, n_off, n_sz)
                for kb in range(9):
                    rtt = rtf(kb)
                    st, sp = (kb == 0), (kb == 8)
                    nc.tensor.matmul(pz0[:], h1_sb[:, kb * W1: kb * W1 + 128], rtt,
                                     start=st, stop=sp)
                    nc.tensor.matmul(pz1[:], h1_sb[:, kb * W1 + 128:(kb + 1) * W1],
                                     rtt, start=st, stop=sp)
                ep_z1(0, 0, 128, n_off, n_sz, pz0)
                ep_z1(1, 128, 128, n_off, n_sz, pz1)
            z1_full = allgather(z1_sl, [H1, NP], "z1_full")

            # ============ J: z1pass [zc-z1 | h2-z1] ============
            W2 = H2 // 8

            def ep_z1pass(mi, m_off, m_sz, n_off, n_sz, psum):
                if mi == 0:
                    nc.vector.tensor_add(zc_acc[:, n_off:n_off + n_sz],
                                         zc_acc[:, n_off:n_off + n_sz], psum[:])
                else:
                    nc.vector.tensor_copy(hT_tmp[0:128, n_off:n_off + n_sz], psum[:])
            tp_gemm(P, sb, ps, [(w_z1p, H1)], [(z1_full, H1)], 256, ep_z1pass)
            nc.vector.tensor_copy(hT_tmp[:, N:NP], zpad[:])
            h2_sb = sb.tile([128, 9 * W2], BF16, name="h2_sb")
            for kb in range(9):
                transpose_block(hT_tmp[0:128, kb * 128:(kb + 1) * 128], [128, 128],
                                h2_sb[:, kb * W2:(kb + 1) * W2])

            # ============ K: z2 agg ============
            z2_sl = local("z2_sl", [H2 // 8, NP])
            zero_pads(z2_sl, H2 // 8)
            bt_g2 = load_bias(P, sb, b_g2, H2 // 8)
            ep_z2 = act_epilogue(P, sb, z2_sl, bt_g2, AF.Relu)
            for (n_off, n_sz) in NCF:
                pz = ps.tile([128, n_sz], F32, name=P.name("pz2"), tag="pA", bufs=2)
                rtf = stream3(ahatT, n_off, n_sz)
                for kb in range(9):
                    rtt = rtf(kb)
                    nc.tensor.matmul(pz[:], h2_sb[:, kb * W2:(kb + 1) * W2], rtt,
                                     start=(kb == 0), stop=(kb == 8))
                ep_z2(0, 0, 128, n_off, n_sz, pz)
            z2_full = allgather(z2_sl, [H2, NP], "z2_full")

            # ============ L: zc final + z0 ============
            zcz0_sl = local("zcz0_sl", [H2 // 8, NP])
            zero_pads(zcz0_sl, H2 // 8)
            bt_ec2 = load_bias(P, sb, b_ec2, H2 // 8)

            def ep_zc(mi, m_off, m_sz, n_off, n_sz, psum):
                t = sb.tile([m_sz, n_sz], F32R, name=P.name("epc"), tag="ep", bufs=3)
                nc.vector.tensor_add(t[:], psum[:], zc_acc[:, n_off:n_off + n_sz])
                nc.scalar.activation(t[:], t[:], AF.Relu, bias=bt_ec2[0:m_sz, 0:1])
                tb = sb.tile([m_sz, n_sz], BF16, name=P.name("epcb"), tag="ep", bufs=3)
                nc.vector.tensor_add(tb[:], t[:], z0_sb[0:m_sz, n_off:n_off + n_sz])
                nc.sync.dma_start(zcz0_sl[m_off:m_off + m_sz, n_off:n_off + n_sz],
                                  tb[:])
            tp_gemm(P, sb, ps, [(w_ec2c, H2)], [(z2_full, H2)], H2 // 8, ep_zc)
            zcz0_full = allgather(zcz0_sl, [H2, NP], "zcz0_full")

            # ============ M: zf ============
            zf_sl = local("zf_sl", [OUTS // 8, NP])
            zero_pads(zf_sl, OUTS // 8)
            bt_ec3 = load_bias(P, sb, b_ec3, OUTS // 8)
            tp_gemm(P, sb, ps, [(w_ec3, H2)], [(zcz0_full, H2)], OUTS // 8,
                    act_epilogue(P, sb, zf_sl, bt_ec3, AF.Relu))
            zf_full = allgather(zf_sl, [OUTS, NP], "zf_full")

            # ============ N: out ============
            bt_out = load_bias(P, sb, b_out, NL)

            def ep_out(mi, m_off, m_sz, n_off, n_sz, psum):
                t = sb.tile([m_sz, n_sz], F32, name=P.name("epo"), tag="ep", bufs=3)
                nc.vector.tensor_scalar_add(t[:], psum[:], bt_out[0:m_sz, mi:mi + 1])
                nc.sync.dma_start(outT[m_off:m_off + m_sz, n_off:n_off + n_sz], t[:])
            tp_gemm(P, sb, ps, [(w_out, OUTS)], [(zf_full, OUTS)], NL, ep_out)
            nc.sync.dma_start(outT[0:NL, N:NP], zrow_f[0:NL, :])

    nc.compile()
    legalize_matmul_waits(nc)
    return nc


def shard_inputs(x, gx, edge_index, jw1, jb1, jw2, jb2, ec1_w, ec1_b, dr_w, dr_b,
                 g1_w, g1_b, g2_w, g2_b, ec2_w, ec2_b, ec3_w, ec3_b, out_w, out_b):
    f32 = np.float32
    bf = ml_dtypes.bfloat16
    x = np.asarray(x); gx = np.asarray(gx)
    xp = np.zeros((NP, DX), f32); xp[:N] = x
    gxp = np.zeros((NP, DX), f32); gxp[:N] = gx
    xgT = np.concatenate([xp.T, gxp.T], axis=0).astype(bf)

    row, col = np.asarray(edge_index[0]), np.asarray(edge_index[1])
    deg = np.bincount(col, minlength=N).astype(f32) + 1.0
    dinv = (1.0 / np.sqrt(deg)).astype(f32)
    ahT = np.zeros((NP, NP), f32)
    np.add.at(ahT, (row, col), (dinv[row] * dinv[col]).astype(f32))
    ahT[np.arange(N), np.arange(N)] += dinv * dinv
    ahT = ahT.astype(bf)

    jw2p = np.zeros((JH, NP), f32); jw2p[:, :N] = jw2
    jb2p = np.zeros((NP,), f32); jb2p[:N] = jb2
    vmask = np.zeros((1, NP), f32); vmask[0, :N] = 1.0

    def cseg(w, c, width):
        return w[:, c * width:(c + 1) * width]

    ins = []
    for c in range(NCORES):
        cs = slice(c * S, (c + 1) * S)
        eyeT = np.zeros((S, NP), f32)
        rr = np.arange(c * S, min((c + 1) * S, N))
        eyeT[rr - c * S, rr] = 1.0
        w_zp = np.concatenate([cseg(dr_w, c, H2 // 8),
                               cseg(ec2_w[:DX], c, H2 // 8),
                               cseg(g1_w, c, H1 // 8)], axis=1)
        w_z1p = np.concatenate([cseg(ec2_w[DX:DX + H1], c, H2 // 8),
                                cseg(g2_w, c, H2 // 8)], axis=1)
        dbf = dict(
            xgT=xgT,
            ahatT=ahT,
            w_jw1=cseg(jw1, c, JH // 8),
            w_jw2=jw2p[:, cs],
            w_ec1x=cseg(ec1_w[:DX], c, H0 // 8),
            w_ec1g=cseg(ec1_w[DX:], c, H0 // 8),
            w_zp=w_zp,
            w_g1gx=cseg(g1_w, c, H1 // 8),
            w_z1p=w_z1p,
            w_ec2c=cseg(ec2_w[DX + H1:], c, H2 // 8),
            w_ec3=cseg(ec3_w, c, OUTS // 8),
            w_out=out_w,
        )
        dft = dict(
            eyeT=eyeT,
            vmask=vmask,
            b_jb1=np.asarray(jb1)[c * (JH // 8):(c + 1) * (JH // 8)].reshape(-1, 1),
            b_jb2=jb2p[cs].reshape(-1, 1),
            b_ec1=np.asarray(ec1_b)[c * (H0 // 8):(c + 1) * (H0 // 8)].reshape(-1, 1),
            b_dr=np.asarray(dr_b)[c * (H2 // 8):(c + 1) * (H2 // 8)].reshape(-1, 1),
            b_g1=np.asarray(g1_b)[c * (H1 // 8):(c + 1) * (H1 // 8)].reshape(-1, 1),
            b_g2=np.asarray(g2_b)[c * (H2 // 8):(c + 1) * (H2 // 8)].reshape(-1, 1),
            b_ec2=np.asarray(ec2_b)[c * (H2 // 8):(c + 1) * (H2 // 8)].reshape(-1, 1),
            b_ec3=np.asarray(ec3_b)[c * (OUTS // 8):(c + 1) * (OUTS // 8)].reshape(-1, 1),
            b_out=np.asarray(out_b).reshape(-1, 1),
            identR=np.eye(128, dtype=f32),
            onescol=np.ones((128, 1), f32),
            onesrow=np.ones((1, 128), f32),
        )
        d = {k: np.ascontiguousarray(np.asarray(v), dtype=bf) for k, v in dbf.items()}
        d.update({k: np.ascontiguousarray(v, dtype=f32) for k, v in dft.items()})
        ins.append(d)
    return ins


_PROG = [None]


def kernel(**inputs) -> np.ndarray:
    in_maps = shard_inputs(**inputs)
    if _PROG[0] is None:
        _PROG[0] = build_program()
    nc = _PROG[0]
    res = bass_utils.run_bass_kernel_spmd(nc, in_maps, core_ids=list(range(NCORES)))
    outT = res.results[0]["outT"]
    return np.ascontiguousarray(outT[:, :N].T)
